# revision 24
# baseline (speedup 1.0000x reference)
"""Causal multi-head self-attention with RoPE on 8 TRN2 NeuronCores.

Problem (hardcoded): B=2, S=2048, D=1024, H=16, d_k=64, fp32 I/O.

Sharding (data + tensor parallel, per the head-group hint):
  core c in 0..7 -> batch b = c//4, head group g = c%4 (4 heads = 256 dims).
  Wq/Wk/Wv split column-wise (by output head dims), Wo split row-wise.
  Each core computes a partial [S, D] output (bf16); the host sums the 4
  partials per batch (row-parallel unshard) in fp32.

v3 design notes:
  - PE matmul stream time dominates; ALL matmuls use the uniform (128,128)
    tile mode to avoid tile-mode-switch drains: scores lhsT tiles are
    zero-padded to K=128 (ktz per head keeps the other head's rows at 0).
  - Causal N-restriction: diagonal score matmuls only compute the valid
    [c*128:512] column range; exp of the last unit reads [256:512] only.
  - Causal mask folded into the scores PSUM accumulation as a second matmul
    (lhsT=strict-upper-tri ones, rhs=-1e9*I) instead of DVE multiplies.
  - ACT does exp (plus V evictions which fit under the PE roofline); all
    other PSUM evictions on DVE; GPSIMD only does partition_broadcast.
  - Softmax denominator via ones-column in V (row 64 of PV PSUM); recip on
    DVE at partition 64, DMA partition-shift to 0, GPSIMD broadcast, fused
    normalize mul.
  - Per-512-chunk tiles (qt/ktz/vau/vals) for fine-grained deps, plus a
    software-pipelined emission schedule (projections injected one stage
    ahead, outproj displaced one stage) so projections, attention, output
    projection and DMAs pipeline end-to-end without PE head-of-line stalls.
"""

import numpy as np
import ml_dtypes

B, S, D = 2, 2048, 1024
H, DK = 16, 64
HPC = 4          # heads per core
E = HPC * DK     # 256 output dims per core
P = 128
KS = D // P      # 8 contraction subtiles
SQT = 512        # sq column width
NJ = S // SQT    # 4 sq columns
NSK = S // P     # 16 sk tiles
BF = ml_dtypes.bfloat16

_CACHE = {}


def _build_nc():
    import concourse.bacc as bacc
    import concourse.mybir as mybir
    import concourse.tile as tile
    from contextlib import ExitStack

    bf = mybir.dt.bfloat16
    f32 = mybir.dt.float32
    Exp = mybir.ActivationFunctionType.Exp

    nc = bacc.Bacc("TRN2", target_bir_lowering=False)

    xT = nc.dram_tensor("xT", [P, NJ, KS, SQT], bf, kind="ExternalInput")
    wq = nc.dram_tensor("wq", [P, KS, E], bf, kind="ExternalInput")
    wk = nc.dram_tensor("wk", [P, KS, E], bf, kind="ExternalInput")
    wv = nc.dram_tensor("wv", [P, KS, E], bf, kind="ExternalInput")
    wo = nc.dram_tensor("wo", [P, 2, D], bf, kind="ExternalInput")
    cs = nc.dram_tensor("cs", [P, S], bf, kind="ExternalInput")
    sn = nc.dram_tensor("sn", [P, S], f32, kind="ExternalInput")
    swp = nc.dram_tensor("swp", [P, P], bf, kind="ExternalInput")
    lt = nc.dram_tensor("lt", [P, P], bf, kind="ExternalInput")
    negI = nc.dram_tensor("negI", [P, P], bf, kind="ExternalInput")
    out = nc.dram_tensor("out", [S, D], f32, kind="ExternalOutput")

    with tile.TileContext(nc) as tc, ExitStack() as ctx:
        const = ctx.enter_context(tc.tile_pool(name="const", bufs=1))
        work = ctx.enter_context(tc.tile_pool(name="work", bufs=2))
        pexp_pool = ctx.enter_context(tc.tile_pool(name="pexpp", bufs=6))
        mm = ctx.enter_context(tc.tile_pool(name="mm", bufs=2, space="PSUM"))
        stp_pool = ctx.enter_context(tc.tile_pool(name="stp", bufs=1, space="PSUM"))
        pv_pool = ctx.enter_context(tc.tile_pool(name="pvp", bufs=1, space="PSUM"))

        # preload the exp table set during the DMA lead-in
        warm = const.tile([1, 1], f32, tag="warm")
        nc.vector.memset(warm[:], 0.0)
        nc.scalar.activation(out=warm[:], in_=warm[:], func=Exp)

        # ---- input DMAs, ordered by first use ----
        wq_sb = const.tile([P, KS, E], bf, tag="wq")
        nc.sync.dma_start(wq_sb[:], wq[:])
        xss = []
        for st in range(NJ):
            xc = const.tile([P, KS, SQT], bf, tag=f"xs{st}", name=f"xs{st}")
            xss.append(xc)
        nc.sync.dma_start(xss[0][:], xT[:, 0, :, :])
        wk_sb = const.tile([P, KS, E], bf, tag="wk")
        nc.sync.dma_start(wk_sb[:], wk[:])
        swp_sb = const.tile([P, P], bf, tag="swp")
        nc.sync.dma_start(swp_sb[:], swp[:])
        cs_sb = const.tile([P, S], bf, tag="cs")
        nc.sync.dma_start(cs_sb[:], cs[:])
        sn_sb = const.tile([P, S], f32, tag="sn")
        nc.sync.dma_start(sn_sb[:], sn[:])
        wv_sb = const.tile([P, KS, E], bf, tag="wv")
        nc.sync.dma_start(wv_sb[:], wv[:])
        lt_sb = const.tile([P, P], bf, tag="lt")
        nc.sync.dma_start(lt_sb[:], lt[:])
        negI_sb = const.tile([P, P], bf, tag="negI")
        nc.sync.dma_start(negI_sb[:], negI[:])
        nc.sync.dma_start(xss[1][:], xT[:, 1, :, :])
        nc.sync.dma_start(xss[2][:], xT[:, 2, :, :])
        nc.sync.dma_start(xss[3][:], xT[:, 3, :, :])
        wo_sb = const.tile([P, 2, D], bf, tag="wo")
        nc.sync.dma_start(wo_sb[:], wo[:])

        # per-chunk persistent tiles for fine-grained deps
        qts = [[const.tile([P, SQT], bf, tag=f"qt{pr}{st}", name=f"qt{pr}{st}")
                for st in range(NJ)] for pr in range(2)]
        # zero-padded K tiles: ktz[pair][par] has K rows at [64par:64par+64],
        # zeros elsewhere, so scores matmuls run K=128 in the uniform mode
        ktz = [[[const.tile([P, SQT], bf, tag=f"kt{pr}{par}{st}",
                            name=f"kt{pr}{par}{st}") for st in range(NJ)]
                for par in range(2)] for pr in range(2)]
        for pr in range(2):
            for st in range(NJ):
                nc.gpsimd.memset(ktz[pr][0][st][64:128, :], 0.0)
                nc.gpsimd.memset(ktz[pr][1][st][0:64, :], 0.0)
        # V augmented with a ones column per head, one tile per sk-tile
        vau = [const.tile([P, HPC * (DK + 1)], bf, tag=f"va{i}", name=f"va{i}")
               for i in range(NSK)]
        for i in range(NSK):
            v4 = vau[i].rearrange("p (h e) -> p h e", h=HPC)
            nc.gpsimd.memset(v4[:, :, DK], 1.0)
        # normalized attention values per j-column, laid out as Wo lhsT
        valst = [const.tile([P, 2, SQT], bf, tag=f"vals{j}", name=f"vals{j}")
                 for j in range(NJ)]

        def project_T(w_sb, pair, st, is_q):
            """rope( (W.T @ x.T)[e-block pair, sq-chunk st] ) -> qts / ktz."""
            sl = slice(st * SQT, (st + 1) * SQT)
            ps = mm.tile([P, SQT], f32, tag="mm")
            for ks in range(KS):
                nc.tensor.matmul(
                    ps[:], lhsT=w_sb[:, ks, pair * P:(pair + 1) * P],
                    rhs=xss[st][:, ks, :], start=(ks == 0), stop=(ks == KS - 1))
            q0 = work.tile([P, SQT], bf, tag="q0")
            nc.vector.tensor_copy(out=q0[:], in_=ps[:])
            tq = work.tile([P, SQT], bf, tag="tq")
            nc.vector.tensor_mul(out=tq[:], in0=q0[:], in1=cs_sb[:, sl])
            psw = mm.tile([P, SQT], f32, tag="mm")
            nc.tensor.matmul(psw[:], lhsT=swp_sb[:], rhs=q0[:],
                             start=True, stop=True)
            tu = work.tile([P, SQT], bf, tag="tu")
            nc.vector.tensor_mul(out=tu[:], in0=psw[:], in1=sn_sb[:, sl])
            if is_q:
                nc.vector.tensor_add(out=qts[pair][st][:], in0=tq[:], in1=tu[:])
            else:
                nc.vector.tensor_add(out=ktz[pair][0][st][0:64, :],
                                     in0=tq[0:64, :], in1=tu[0:64, :])
                nc.vector.tensor_add(out=ktz[pair][1][st][64:128, :],
                                     in0=tq[64:128, :], in1=tu[64:128, :])

        def project_v(sst):
            ps = mm.tile([P, SQT], f32, tag="mm")
            pv256 = ps[:, 0:E]
            for ks in range(KS):
                nc.tensor.matmul(
                    pv256,
                    lhsT=xss[sst // 4][:, ks, (sst % 4) * P:(sst % 4 + 1) * P],
                    rhs=wv_sb[:, ks, :], start=(ks == 0), stop=(ks == KS - 1))
            v4 = vau[sst].rearrange("p (h e) -> p h e", h=HPC)
            nc.vector.tensor_copy(out=v4[:, :, 0:DK],
                               in_=pv256.rearrange("p (h e) -> p h e", h=HPC))

        def attention(pair, j, items=()):
            items = list(items)
            last_i = 4 * j + 3
            pvt = pv_pool.tile([P, 2, SQT], f32, tag="pv", name="pv")
            pvts = [pvt[:, 0, :], pvt[:, 1, :]]
            for g in range(2 * j + 2):
                narrow = (g == 2 * j + 1)
                win = 256 if narrow else 0
                stps = [stp_pool.tile([P, 2, SQT], f32, tag=f"st{par}",
                                      name=f"st{par}") for par in range(2)]
                for c2 in range(2):
                    i = 2 * g + c2
                    c = i - 4 * j
                    noff = c * P if c > 0 else 0
                    for par in range(2):
                        nc.tensor.matmul(
                            stps[par][:, c2, noff:],
                            lhsT=ktz[pair][par][i // 4][:, (i % 4) * P:(i % 4 + 1) * P],
                            rhs=qts[pair][j][:, noff:],
                            start=True, stop=(c < 0))
                    if c >= 0:
                        for par in range(2):
                            nc.tensor.matmul(
                                stps[par][:, c2, c * P:(c + 1) * P],
                                lhsT=lt_sb[:], rhs=negI_sb[:],
                                start=False, stop=True)
                if g < len(items) and items[g] is not None:
                    items[g]()
                for par in range(2):
                    hl = 2 * pair + par
                    pexp = pexp_pool.tile([P, 2, SQT], bf, tag="pexp", name="pexp")
                    if g >= 2 * j:
                        # diagonal unit: exp exactly the written ranges per c2
                        for c2 in range(2):
                            c = 2 * g + c2 - 4 * j
                            noff = c * P if c > 0 else 0
                            nc.scalar.activation(out=pexp[:, c2, noff:],
                                                 in_=stps[par][:, c2, noff:],
                                                 func=Exp)
                    else:
                        nc.scalar.activation(out=pexp[:, :, win:],
                                             in_=stps[par][:, :, win:], func=Exp)
                    for c2 in range(2):
                        i = 2 * g + c2
                        c = i - 4 * j
                        off = c * P if c > 0 else 0
                        nc.tensor.matmul(
                            pvt[0:DK + 1, par, off:SQT],
                            lhsT=vau[i][:, hl * (DK + 1):(hl + 1) * (DK + 1)],
                            rhs=pexp[:, c2, off:SQT],
                            start=(i == 0), stop=(i == last_i))
            for it in items[2 * j + 2:]:
                if it is not None:
                    it()
            # normalize by softmax denominator (row DK of pvt): one
            # cross-partition copy + one reciprocal covers both heads,
            # then per-head broadcast + fused normalize mul (par1 first —
            # its chain is longer: staging + partition-shift DMA)
            dn = work.tile([P, 2, SQT], f32, tag="dn", name="dn")
            rs = work.tile([P, 2, SQT], f32, tag="rs", name="rs")
            nc.scalar.copy(out=dn[0:1, :, :], in_=pvt[DK:DK + 1, :, :])
            nc.vector.reciprocal_approx_fast(out=rs[0:1, :, :],
                                             in_=dn[0:1, :, :])
            for par in (1, 0):
                rb = work.tile([P, SQT], f32, tag="rb", name="rb")
                nc.gpsimd.partition_broadcast(rb[0:DK, :], rs[0:1, par, :],
                                              channels=DK)
                if par == 0:
                    nc.vector.tensor_mul(out=valst[j][0:DK, pair, :],
                                         in0=pvts[par][0:DK, :], in1=rb[0:DK, :])
                else:
                    stg = work.tile([P, SQT], bf, tag="stg")
                    nc.vector.tensor_mul(out=stg[0:DK, :],
                                         in0=pvts[par][0:DK, :], in1=rb[0:DK, :])
                    nc.sync.dma_start(out=valst[j][DK:2 * DK, pair, :],
                                      in_=stg[0:DK, :])

        def outproj(j, late=False):
            for sq4 in range(4):
                sq = 4 * j + sq4
                ostg = work.tile([P, 2, SQT], f32, tag="ostg", name="ostg")
                for n2 in range(2):
                    ps = mm.tile([P, SQT], f32, tag="mm")
                    for ks2 in range(2):
                        nc.tensor.matmul(
                            ps[:],
                            lhsT=valst[j][:, ks2, sq4 * P:(sq4 + 1) * P],
                            rhs=wo_sb[:, ks2, n2 * SQT:(n2 + 1) * SQT],
                            start=(ks2 == 0), stop=(ks2 == 1))
                    if late and n2 == 0:
                        nc.scalar.copy(out=ostg[:, n2, :], in_=ps[:])
                    else:
                        nc.vector.tensor_copy(out=ostg[:, n2, :], in_=ps[:])
                nc.sync.dma_start(out=out[sq * P:(sq + 1) * P, :], in_=ostg[:])

        # ---- software-pipelined schedule: projections run one attention
        # stage ahead (injected after the first g-unit of each column) and
        # outproj is displaced one stage so it never head-of-line-blocks the
        # PE queue on the normalize chain ----
        def pq(pair, st):
            return lambda: project_T(wq_sb, pair, st, True)

        def pk(pair, st):
            return lambda: project_T(wk_sb, pair, st, False)

        def pv_(sst):
            return lambda: project_v(sst)

        # pair-0 then pair-1, ascending; projections injected one stage
        # ahead; outproj displaced late into the following column so the
        # normalize chain never head-of-line-blocks the PE queue.
        pre = [pq(0, 0), pk(0, 0), pv_(0), pv_(1), pv_(2), pv_(3)]
        for it in pre:
            it()
        plan = [
            ((0, 0), [pq(0, 1), pk(0, 1), pv_(4), pv_(5), pv_(6), pv_(7)]),
            ((0, 1), [pq(0, 2), pk(0, 2), pv_(8), pv_(9), pv_(10), pv_(11)]),
            ((0, 2), [pq(0, 3), pk(0, 3), pv_(12), pv_(13), pv_(14), pv_(15)]),
            ((0, 3), [pq(1, 0), pk(1, 0), pq(1, 1), pk(1, 1)]),
            ((1, 0), [pq(1, 2), pk(1, 2)]),
            ((1, 1), [pq(1, 3), pk(1, 3), None, lambda: outproj(0)]),
            ((1, 2), [None, None, None, None, None, lambda: outproj(1)]),
            ((1, 3), [None, None, None, None, None, None, None,
                      lambda: outproj(2)]),
        ]
        for (pair, st), items in plan:
            attention(pair, st, items)
        outproj(3, late=True)

    nc.compile()
    return nc


def get_nc():
    if "nc" not in _CACHE:
        _CACHE["nc"] = _build_nc()
    return _CACHE["nc"]


def make_in_maps(x, Wq, Wk, Wv, Wo, token_positions, rope_theta):
    """Host-side sharding: per-core input dict (bf16, pre-transposed/permuted)."""
    x = np.asarray(x, np.float32)
    Wq = np.asarray(Wq, np.float32)
    Wk = np.asarray(Wk, np.float32)
    Wv = np.asarray(Wv, np.float32)
    Wo = np.asarray(Wo, np.float32)
    pos = np.asarray(token_positions).astype(np.float32)
    theta = float(np.asarray(rope_theta))

    perm = np.concatenate([np.arange(0, DK, 2), np.arange(1, DK, 2)])  # evens, odds
    freqs = theta ** (-np.arange(DK // 2, dtype=np.float32) / (DK // 2))
    ang = pos[:, None] * freqs[None, :]          # [S, 32]
    cosT = np.cos(ang).T.astype(np.float32)      # [32, S]
    sinT = np.sin(ang).T.astype(np.float32)
    cs_t = np.tile(cosT, (4, 1)).astype(BF)                          # [128, S]
    sn_t = np.concatenate([-sinT, sinT, -sinT, sinT], 0).astype(np.float32)  # [128, S]

    sigma = np.arange(P)
    sigma = np.where((sigma // 32) % 2 == 0, sigma + 32, sigma - 32)
    swp_t = np.zeros((P, P), np.float32)
    swp_t[sigma, np.arange(P)] = 1.0
    swp_t = swp_t.astype(BF)

    lt_t = np.triu(np.ones((P, P), np.float32), 1).astype(BF)  # lt[k,p]=k<p
    negI_t = (np.eye(P, dtype=np.float32) * -1e9).astype(BF)

    in_maps = []
    for c in range(8):
        b, g = c // 4, c % 4
        hs = slice(g * E, (g + 1) * E)

        def prep_qk(W, scale):
            Wl = W[hs].reshape(HPC, DK, D)[:, perm, :].reshape(E, D) * scale
            return np.ascontiguousarray(Wl.T).astype(BF)

        def to_pke(w):  # [D, E] -> [P, KS, E] partition-contiguous
            return np.ascontiguousarray(w.reshape(KS, P, -1).transpose(1, 0, 2))

        xr = x[b].T.reshape(KS, P, NJ, SQT).transpose(1, 2, 0, 3)
        wor = Wo[:, hs].T.reshape(2, P, D).transpose(1, 0, 2)
        in_maps.append({
            "xT": np.ascontiguousarray(xr).astype(BF),
            "wq": to_pke(prep_qk(Wq, 1.0 / np.sqrt(DK))),
            "wk": to_pke(prep_qk(Wk, 1.0)),
            "wv": to_pke(np.ascontiguousarray(Wv[hs].T).astype(BF)),
            "wo": np.ascontiguousarray(wor).astype(BF),
            "cs": cs_t, "sn": sn_t, "swp": swp_t, "lt": lt_t, "negI": negI_t,
        })
    return in_maps


def kernel(x, Wq, Wk, Wv, Wo, token_positions, rope_theta):
    nc = get_nc()
    in_maps = make_in_maps(x, Wq, Wk, Wv, Wo, token_positions, rope_theta)
    from concourse.bass_utils import run_bass_kernel_spmd
    r = run_bass_kernel_spmd(nc, in_maps, core_ids=list(range(8)))
    outs = [np.asarray(m["out"], np.float32) for m in r.results]
    full = np.stack([sum(outs[0:4]), sum(outs[4:8])], 0)
    return full.astype(np.float32)
